# revision 26
# baseline (speedup 1.0000x reference)
"""Trainium2 Bass kernel for nn_AutoregressiveArithmeticTransformer.

6-layer dense transformer: B=16, T=512, E=512, NH=8 heads x HS=64, FF=2048,
V=16, causal attention, pre-LN, learned abacus embedding, logits / 0.8.

Strategy: data-parallel over batch across 8 NeuronCores (2 sequences per
core, no collectives). Activations feature-major in SBUF; the residual
stream is bf16. All heavy matmuls (QKV/V/proj/FFN and the attn o/den
matmuls) run in fp8-e4m3 with the TensorE DoubleRow perf mode (2 k-tiles
of 128 per instruction, 2x bf16 throughput); weights are pre-scaled
host-side (x64 / x32) to sit in the fp8 normal range, and the inverse
scales are folded into downstream PSUM-eviction ops / the softmax exp
scale. Scores matmuls stay bf16 (K=64 cannot use DoubleRow). LayerNorm
statistics are ones-matmuls on the PE in bf16; softmax denominators are
ones-matmuls in fp8 sharing the causal block structure of the o matmul.
"""

import numpy as np
import ml_dtypes

import concourse.bacc as bacc
import concourse.tile as tile
from concourse import mybir

F32 = mybir.dt.float32
BF16 = mybir.dt.bfloat16
FP8 = mybir.dt.float8e4
AF = mybir.ActivationFunctionType
OP = mybir.AluOpType
DR = mybir.MatmulPerfMode.DoubleRow

E4 = ml_dtypes.float8_e4m3

# Model constants (hardcoded per contest contract)
V, E, NH, HS, FF, NB, L = 16, 512, 8, 64, 2048, 6, 512
B, T = 16, 512
TEMP = 1.0 * 0.8
EPS = 1e-5
SCALE = HS ** -0.5  # 0.125

NCORES = 8
SEQ = 2              # sequences per core
NTOK = SEQ * T       # 1024 tokens per core
C = E // 128         # 4 E-chunks
CF = FF // 128       # 16 FF-chunks
HP = NH // 2         # 4 head-pairs
NJ = T // 128        # 4 tk chunks per sequence

# fp8 weight pre-scales (keep weights in the e4m3 normal range)
WS_QK = 64.0         # wq, wk
WS_V = 64.0          # wv
WS_PW = 64.0         # proj_w
WS_F1 = 32.0         # ff_w1
WS_F2 = 64.0         # ff_w2
VT_SCALE = 8.0                 # vt8 = 8 * v_true
EXP_SCALE = SCALE / (WS_QK * WS_QK)
PROJ_RSCALE = 1.0 / (8.0 * WS_PW)      # o8 carries 8x
FFN_RSCALE = 1.0 / (WS_F1 * WS_F2)     # fa8 carries 32x

_PROGRAM_CACHE = {}


def _ceil2(x):
    return (x + 1) // 2 * 2


def build_program(ln_trivial, pb_zero, fb2_zero, nb_run=NB, dbg=False):
    """ln_trivial: list of 2*NB+1 bools; pb_zero/fb2_zero: biases all-zero."""
    nc = bacc.Bacc(None, target_bir_lowering=False)

    h0_d = nc.dram_tensor("h0", [128, C * NTOK], BF16, kind="ExternalInput")
    wq_d = nc.dram_tensor("wq", [NB, 128, C * 512], FP8, kind="ExternalInput")
    wk_d = nc.dram_tensor("wk", [NB, 128, C * 512], FP8, kind="ExternalInput")
    wv_d = nc.dram_tensor("wv", [NB, 128, C * 512], BF16, kind="ExternalInput")
    pw_d = nc.dram_tensor("pw", [NB, 128, C * 512], BF16, kind="ExternalInput")
    f1_d = nc.dram_tensor("f1", [NB, 128, C * FF], BF16, kind="ExternalInput")
    f2_d = nc.dram_tensor("f2", [NB, 128, CF * 512], BF16, kind="ExternalInput")
    pb_d = nc.dram_tensor("pb", [128, NB * C], F32, kind="ExternalInput")
    fb1_d = nc.dram_tensor("fb1", [128, NB * CF], F32, kind="ExternalInput")
    fb2_d = nc.dram_tensor("fb2", [128, NB * C], F32, kind="ExternalInput")
    ow_d = nc.dram_tensor("ow", [128, C * V], BF16, kind="ExternalInput")
    ob_d = nc.dram_tensor("ob", [V, 1], F32, kind="ExternalInput")
    tri_d = nc.dram_tensor("tri", [128, 128], BF16, kind="ExternalInput")
    lng_d = nc.dram_tensor("lng", [128, (2 * NB + 1) * C], F32,
                           kind="ExternalInput")
    lnb_d = nc.dram_tensor("lnb", [128, (2 * NB + 1) * C], F32,
                           kind="ExternalInput")
    out_d = nc.dram_tensor("logits", [V, NTOK], F32, kind="ExternalOutput")
    if dbg:
        xn_dbg_d = nc.dram_tensor("xn_dbg", [128, C * NTOK], F32,
                                  kind="ExternalOutput")
        o_dbg_d = nc.dram_tensor("o_dbg", [128, C * NTOK], F32,
                                 kind="ExternalOutput")
        h1_dbg_d = nc.dram_tensor("h1_dbg", [128, C * NTOK], F32,
                                  kind="ExternalOutput")
        vt_dbg_d = nc.dram_tensor("vt_dbg", [128, SEQ * NJ * NH * 128], FP8,
                                  kind="ExternalOutput")
        p_dbg_d = nc.dram_tensor("p_dbg", [128, 2 * NJ * 512], FP8,
                                 kind="ExternalOutput")
        ops_dbg_d = nc.dram_tensor("ops_dbg", [128, 2 * 512], F32,
                                   kind="ExternalOutput")

    from contextlib import ExitStack
    with ExitStack() as ctx:
        tc = ctx.enter_context(tile.TileContext(nc))
        consts = ctx.enter_context(tc.tile_pool(name="consts", bufs=1))
        hpool = ctx.enter_context(tc.tile_pool(name="hpool", bufs=1))
        wqkv = ctx.enter_context(tc.tile_pool(name="wqkv", bufs=2))
        wff1 = ctx.enter_context(tc.tile_pool(name="wff1", bufs=1 if dbg else 2))
        wff2 = ctx.enter_context(tc.tile_pool(name="wff2", bufs=1 if dbg else 2))
        xnp = ctx.enter_context(tc.tile_pool(name="xnp", bufs=2))
        sqp = ctx.enter_context(tc.tile_pool(name="sqp", bufs=1))
        qkp = ctx.enter_context(tc.tile_pool(name="qkp", bufs=2))
        pp = ctx.enter_context(tc.tile_pool(name="pp", bufs=3))
        osb = ctx.enter_context(tc.tile_pool(name="osb", bufs=1))
        ffa = ctx.enter_context(tc.tile_pool(name="ffa", bufs=1))
        stats = ctx.enter_context(tc.tile_pool(name="stats", bufs=2))
        lnt = ctx.enter_context(tc.tile_pool(name="lnt", bufs=1))
        dbgp = ctx.enter_context(tc.tile_pool(name="dbgp", bufs=1)) if dbg \
            else None
        rdp = ctx.enter_context(tc.tile_pool(name="rdp", bufs=1))
        # PSUM: ps1 = 1-bank [128,512] tiles; ps2 = 2-bank [128,2,512]
        ps1 = ctx.enter_context(tc.tile_pool(name="ps1", bufs=2, space="PSUM"))
        ps2 = ctx.enter_context(tc.tile_pool(name="ps2", bufs=3, space="PSUM"))

        ones_t = consts.tile([128, 128], BF16)
        nc.gpsimd.memset(ones_t[:], 1.0)
        ones8_t = consts.tile([128, 2, 128], FP8)
        nc.gpsimd.memset(ones8_t[:], 8.0)
        eps_t = consts.tile([128, 1], F32)
        nc.gpsimd.memset(eps_t[:], float(EPS))
        tri_t = consts.tile([128, 128], BF16)
        nc.sync.dma_start(tri_t[:], tri_d[:])
        pb_t = consts.tile([128, NB * C], F32)
        nc.sync.dma_start(pb_t[:], pb_d[:])
        fb1_t = consts.tile([128, NB * CF], F32)
        nc.sync.dma_start(fb1_t[:], fb1_d[:])
        fb2_t = consts.tile([128, NB * C], F32)
        nc.sync.dma_start(fb2_t[:], fb2_d[:])
        ow_t = consts.tile([128, C, V], BF16)
        nc.sync.dma_start(ow_t[:], ow_d[:].rearrange("p (c v) -> p c v", v=V))
        ob_t = consts.tile([V, 1], F32)
        nc.sync.dma_start(ob_t[:], ob_d[:])
        lng_t = consts.tile([128, 2 * NB + 1, C], F32)
        nc.sync.dma_start(lng_t[:], lng_d[:].rearrange("p (l c) -> p l c", c=C))
        lnb_t = consts.tile([128, 2 * NB + 1, C], F32)
        nc.sync.dma_start(lnb_t[:], lnb_d[:].rearrange("p (l c) -> p l c", c=C))

        h_t = hpool.tile([128, C, NTOK], BF16)
        # V values augmented with a ones column-block per head: the o-matmul
        # lhsT [v64 | ones64] produces o on partitions 0:64 and the softmax
        # denominator on 64:128 of the same PSUM tile (DoubleRow needs
        # partition-base-0 outputs, so den cannot be tile-positioned).
        vt8aug_bufs = []
        for vb in range(2):
            vt8b = hpool.tile([128, SEQ * NJ, NH, 128], FP8,
                              name=f"vt8b{vb}")
            nc.gpsimd.memset(vt8b[:, :, 0:NH:2, 64:128], 1.0)
            nc.gpsimd.memset(vt8b[:, :, 1:NH:2, 0:64], 1.0)
            vt8aug_bufs.append(vt8b)
        nc.sync.dma_start(h_t[:], h0_d[:].rearrange("p (c t) -> p c t",
                                                    t=NTOK))

        def ln_params(idx):
            if not ln_trivial[idx]:
                return lng_t[:, idx, :], lnb_t[:, idx, :], False
            return None, None, True

        # ---- LayerNorm: one token-tile (512 cols) ----
        def emit_ln_tt(tt, g_ap, b_ap, triv, xn, out_fp8):
            sl = slice(tt * 512, tt * 512 + 512)
            sq = sqp.tile([128, C, 512], BF16, tag="sq", name="sq")
            nc.vector.tensor_tensor(sq[:], h_t[:, :, sl], h_t[:, :, sl],
                                    OP.mult)
            s1 = ps1.tile([128, 512], F32, tag="ps1", name="s1")
            s2 = ps1.tile([128, 512], F32, tag="ps1", name="s2")
            for c in range(C):
                nc.tensor.matmul(s1[:], ones_t[:], h_t[:, c, sl],
                                 start=(c == 0), stop=(c == C - 1))
                nc.tensor.matmul(s2[:], ones_t[:], sq[:, c, :],
                                 start=(c == 0), stop=(c == C - 1))
            m_bf = stats.tile([128, 512], BF16, tag="m", name="m_bf")
            nc.scalar.mul(m_bf[:], s1[:], 1.0 / E)
            msq = stats.tile([128, 512], BF16, tag="msq", name="msq")
            nc.scalar.square(msq[:], m_bf[:])
            var = stats.tile([128, 512], F32, tag="var", name="var")
            nc.vector.scalar_tensor_tensor(out=var[:], in0=s2[:],
                                           scalar=1.0 / E, in1=msq[:],
                                           op0=OP.mult, op1=OP.subtract)
            std = stats.tile([128, 512], F32, tag="std", name="std")
            nc.scalar.activation(std[:], var[:], AF.Sqrt, bias=eps_t[:])
            rc = stats.tile([128, 512], F32, tag="rc", name="rc")
            nc.vector.reciprocal_approx_fast(out=rc[:], in_=std[:])
            t1 = lnt.tile([128, C, 512], BF16, tag="t1", name="t1")
            nc.vector.tensor_tensor(
                t1[:], h_t[:, :, sl],
                m_bf[:, None, :].to_broadcast((128, C, 512)), OP.subtract)
            if triv:
                nc.vector.tensor_tensor(
                    xn[:, :, sl], t1[:],
                    rc[:, None, :].to_broadcast((128, C, 512)), OP.mult)
            else:
                xb = lnt.tile([128, C, 512], BF16, tag="xb", name="xb")
                nc.vector.tensor_tensor(
                    xb[:], t1[:],
                    rc[:, None, :].to_broadcast((128, C, 512)), OP.mult)
                for c in range(C):
                    nc.vector.tensor_scalar(
                        out=xn[:, c, sl], in0=xb[:, c, :],
                        scalar1=g_ap[:, c:c + 1], scalar2=b_ap[:, c:c + 1],
                        op0=OP.mult, op1=OP.add)

        def emit_ln(idx, out_fp8=True):
            g_ap, b_ap, triv = ln_params(idx)
            xn = xnp.tile([128, C, NTOK], BF16, tag="xn", name="xn")
            for tt in range(2):
                emit_ln_tt(tt, g_ap, b_ap, triv, xn, out_fp8)
            return xn

        for i in range(nb_run):
            # ---- load this layer's weights (fp8) ----
            wq_t = wqkv.tile([128, C, 512], FP8, tag="wq", name="wq_t")
            nc.sync.dma_start(wq_t[:], wq_d[i].rearrange(
                "p (c m) -> p c m", m=512))
            wk_t = wqkv.tile([128, C, 512], FP8, tag="wk", name="wk_t")
            nc.sync.dma_start(wk_t[:], wk_d[i].rearrange(
                "p (c m) -> p c m", m=512))
            wv_t = wqkv.tile([128, C, 512], BF16, tag="wv", name="wv_t")
            nc.sync.dma_start(wv_t[:], wv_d[i].rearrange(
                "p (c m) -> p c m", m=512))
            pw_t = wqkv.tile([128, C, 512], BF16, tag="pw", name="pw_t")
            nc.sync.dma_start(pw_t[:], pw_d[i].rearrange(
                "p (c m) -> p c m", m=512))
            f1_t = wff1.tile([128, C, FF], BF16, tag="f1", name="f1_t")
            nc.sync.dma_start(f1_t[:], f1_d[i].rearrange(
                "p (c m) -> p c m", m=FF))
            f2_t = wff2.tile([128, CF, 512], BF16, tag="f2", name="f2_t")
            nc.sync.dma_start(f2_t[:], f2_d[i].rearrange(
                "p (c m) -> p c m", m=512))

            if i == 0:
                xn = emit_ln(0, out_fp8=False)
            else:
                xn = xn_next
            vt8aug = vt8aug_bufs[i % 2]

            # ---- V projection, token-major: vt8aug[tok, head, 0:64] = 8*v
            for jg in range(SEQ * NJ):
                vp = ps1.tile([128, 512], F32, tag="ps1", name="vp")
                for c in range(C):
                    nc.tensor.matmul(vp[:],
                                     xn[:, c, jg * 128:(jg + 1) * 128],
                                     wv_t[:, c, :],
                                     start=(c == 0), stop=(c == C - 1))
                vpr = vp[:].rearrange("p (hp h2 d) -> p hp h2 d",
                                      h2=2, d=64)
                nc.scalar.mul(vt8aug[:, jg, 0:NH:2, 0:64], vpr[:, :, 0, :],
                              VT_SCALE)
                nc.scalar.mul(vt8aug[:, jg, 1:NH:2, 64:128], vpr[:, :, 1, :],
                              VT_SCALE)

            o_t = osb.tile([128, C, NTOK], BF16, tag="o", name="o_t")

            def emit_den_o(s, hp, p_t):
                base = s * T
                # o matmul: lhsT [v|ones] (even head) / [ones|v] (odd) so
                # each head's o lands on its destination partition half
                # (DVE reads are partition-aligned with the out base, and
                # DoubleRow outputs must start at partition 0).
                ops = ps2.tile([128, 2, 512], F32, tag="ps2", name="ops")
                den = ps2.tile([128, 2, 512], F32, tag="ps2", name="den")
                for h2 in range(2):
                    head = hp * 2 + h2
                    for dst, lh in ((ops, lambda j: vt8aug[:, s * NJ + j:
                                                          s * NJ + j + 2,
                                                          head, :]),
                                    (den, lambda j: ones8_t[:])):
                        nc.tensor.matmul(dst[:, h2, :], lh(0),
                                         p_t[:, h2, 0:2, :],
                                         start=True, stop=False,
                                         perf_mode=DR)
                        nc.tensor.matmul(dst[:, h2, 256:512], lh(2),
                                         p_t[:, h2, 2:4, 256:512],
                                         start=False, stop=True,
                                         perf_mode=DR)
                if dbg and s == 0 and hp == 0:
                    opsc = dbgp.tile([128, C, NTOK], F32, tag="dbg", name="opsc")[:, 0:2, 0:512]
                    nc.scalar.copy(opsc, ops[:])
                    nc.sync.dma_start(ops_dbg_d[:].rearrange(
                        "p (a t) -> p a t", a=2), opsc)
                    nc.sync.dma_start(p_dbg_d[:], p_t[:].rearrange(
                        "p a j t -> p (a j t)"))
                    nc.sync.dma_start(vt_dbg_d[:], vt8aug[:].rearrange(
                        "p j h d -> p (j h d)"))
                rd = rdp.tile([128, 2, 512], F32, tag="rd", name="rd")
                nc.vector.reciprocal_approx_fast(out=rd[:], in_=den[:])
                for h2 in range(2):
                    dsl = slice(h2 * 64, h2 * 64 + 64)
                    nc.vector.tensor_tensor(
                        o_t[dsl, hp, base:base + T],
                        ops[dsl, h2, :], rd[dsl, h2, :], OP.mult)

            pending = None
            for hp in range(HP):
                msl = slice(hp * 128, (hp + 1) * 128)
                qk_t = qkp.tile([128, 2, NTOK], BF16, tag="qk", name="qk_t")
                for tt in range(2):
                    sl = slice(tt * 512, tt * 512 + 512)
                    qkps = ps2.tile([128, 2, 512], F32, tag="ps2", name="qkps")
                    for c in range(C):
                        nc.tensor.matmul(qkps[:, 0, :], wq_t[:, c, msl],
                                         xn[:, c, sl],
                                         start=(c == 0), stop=(c == C - 1))
                        nc.tensor.matmul(qkps[:, 1, :], wk_t[:, c, msl],
                                         xn[:, c, sl],
                                         start=(c == 0), stop=(c == C - 1))
                    nc.vector.tensor_copy(qk_t[:, :, sl], qkps[:])

                for s in range(SEQ):
                    base = s * T
                    p_t = pp.tile([128, 2, NJ, 512], FP8, tag="p", name="p_t")
                    nc.vector.memset(p_t[:, :, 1, 0:128], 0.0)
                    nc.vector.memset(p_t[:, :, 3, 256:384], 0.0)
                    for j in range(NJ):
                        off = j * 128
                        njw = T - off
                        sT = ps2.tile([128, 2, 512], F32, tag="ps2",
                                      name="sT")
                        for h2 in range(2):
                            dsl = slice(h2 * 64, h2 * 64 + 64)
                            nc.tensor.matmul(
                                sT[:, h2, off:512],
                                qk_t[dsl, 1, base + off:base + off + 128],
                                qk_t[dsl, 0, base + off:base + T],
                                start=True, stop=True)
                        nc.scalar.activation(
                            p_t[:, :, j, off:512], sT[:, :, off:512],
                            AF.Exp, scale=EXP_SCALE)
                        nc.vector.tensor_tensor(
                            p_t[:, :, j, off:off + 128],
                            p_t[:, :, j, off:off + 128],
                            tri_t[:, None, :].to_broadcast((128, 2, 128)),
                            OP.mult)
                    if pending is not None:
                        emit_den_o(*pending)
                    pending = (s, hp, p_t)
            emit_den_o(*pending)

            if dbg and i == 0:
                dcp = dbgp.tile([128, C, NTOK], F32, tag="dbg", name="dcp")
                nc.scalar.copy(dcp[:], o_t[:])
                nc.sync.dma_start(o_dbg_d[:], dcp[:].rearrange(
                    "p c t -> p (c t)"))

            # ---- attention out projection + residual ----
            for tt in range(2):
                sl = slice(tt * 512, tt * 512 + 512)
                for mcp in range(2):
                    pj = ps2.tile([128, 2, 512], F32, tag="ps2", name="pj")
                    for mc2 in range(2):
                        mc = 2 * mcp + mc2
                        for c in range(C):
                            nc.tensor.matmul(
                                pj[:, mc2, :],
                                pw_t[:, c, mc * 128:(mc + 1) * 128],
                                o_t[:, c, sl],
                                start=(c == 0), stop=(c == C - 1))
                    if pb_zero:
                        nc.vector.scalar_tensor_tensor(
                            out=h_t[:, 2 * mcp:2 * mcp + 2, sl],
                            in0=pj[:], scalar=1.0,
                            in1=h_t[:, 2 * mcp:2 * mcp + 2, sl],
                            op0=OP.mult, op1=OP.add)
                    else:
                        tmp = stats.tile([128, 2, 512], BF16, tag="tmp",
                                         name="tmp")
                        for mc2 in range(2):
                            mc = 2 * mcp + mc2
                            nc.vector.tensor_scalar(
                                out=tmp[:, mc2, :], in0=pj[:, mc2, :],
                                scalar1=1.0,
                                scalar2=pb_t[:, i * C + mc:i * C + mc + 1],
                                op0=OP.mult, op1=OP.add)
                        nc.vector.tensor_tensor(
                            h_t[:, 2 * mcp:2 * mcp + 2, sl], tmp[:],
                            h_t[:, 2 * mcp:2 * mcp + 2, sl], OP.add)

            if dbg and i == 0:
                dcp3 = dbgp.tile([128, C, NTOK], F32, tag="dbg", name="dcp3")
                nc.scalar.copy(dcp3[:], h_t[:])
                nc.sync.dma_start(h1_dbg_d[:], dcp3[:].rearrange(
                    "p c t -> p (c t)"))

            # ---- LN2 + FFN (token-tile split) ----
            xn2 = emit_ln(2 * i + 1, out_fp8=False)

            if i + 1 < nb_run:
                g_next, b_next, triv_next = ln_params(2 * (i + 1))
                xn_next = xnp.tile([128, C, NTOK], BF16, tag="xn",
                                   name="xn_next")

            for tt in range(2):
                sl = slice(tt * 512, tt * 512 + 512)
                fa = ffa.tile([128, CF, 512], BF16, tag="fa", name="fa")
                for mfp in range(CF // 2):
                    fp = ps2.tile([128, 2, 512], F32, tag="ps2", name="fp")
                    for mf2 in range(2):
                        mf = 2 * mfp + mf2
                        for c in range(C):
                            nc.tensor.matmul(
                                fp[:, mf2, :],
                                f1_t[:, c, mf * 128:(mf + 1) * 128],
                                xn2[:, c, sl],
                                start=(c == 0), stop=(c == C - 1))
                    for mf2 in range(2):
                        mf = 2 * mfp + mf2
                        nc.scalar.activation(
                            fa[:, mf, :], fp[:, mf2, :], AF.Relu,
                            bias=fb1_t[:, i * CF + mf:i * CF + mf + 1])
                for mcp in range(2):
                    f2p = ps2.tile([128, 2, 512], F32, tag="ps2", name="f2p")
                    for mc2 in range(2):
                        mc = 2 * mcp + mc2
                        for c16 in range(CF):
                            nc.tensor.matmul(
                                f2p[:, mc2, :],
                                f2_t[:, c16, mc * 128:(mc + 1) * 128],
                                fa[:, c16, :],
                                start=(c16 == 0), stop=(c16 == CF - 1))
                    if fb2_zero:
                        nc.vector.scalar_tensor_tensor(
                            out=h_t[:, 2 * mcp:2 * mcp + 2, sl],
                            in0=f2p[:], scalar=1.0,
                            in1=h_t[:, 2 * mcp:2 * mcp + 2, sl],
                            op0=OP.mult, op1=OP.add)
                    else:
                        tmp = stats.tile([128, 2, 512], BF16, tag="tmp",
                                         name="tmp2")
                        for mc2 in range(2):
                            mc = 2 * mcp + mc2
                            nc.vector.tensor_scalar(
                                out=tmp[:, mc2, :], in0=f2p[:, mc2, :],
                                scalar1=1.0,
                                scalar2=fb2_t[:, i * C + mc:i * C + mc + 1],
                                op0=OP.mult, op1=OP.add)
                        nc.vector.tensor_tensor(
                            h_t[:, 2 * mcp:2 * mcp + 2, sl], tmp[:],
                            h_t[:, 2 * mcp:2 * mcp + 2, sl], OP.add)
                # peel next layer's LN1 token-tile
                if i + 1 < nb_run:
                    emit_ln_tt(tt, g_next, b_next, triv_next, xn_next, True)

        # ---- final LN + logits ----
        xnf = emit_ln(2 * NB if nb_run == NB else 0, out_fp8=False)
        lg_sb = consts.tile([V, NTOK], F32)
        for tt in range(2):
            sl = slice(tt * 512, tt * 512 + 512)
            lg = ps1.tile([V, 512], F32, tag="ps1", name="lg")
            for c in range(C):
                nc.tensor.matmul(lg[:], ow_t[:, c, :], xnf[:, c, sl],
                                 start=(c == 0), stop=(c == C - 1))
            nc.vector.tensor_scalar_add(lg_sb[:, sl], lg[:], ob_t[:])
        nc.sync.dma_start(out_d[:], lg_sb[:])

    nc.finalize()
    return nc


def _to_fp8(x):
    return np.clip(x, -240.0, 240.0).astype(E4)


def prepare_inputs(inputs):
    """Host-side preprocessing: embedding gather, weight layout + fp8 cast.
    Returns (shared_map, per_core_h0_list, flags)."""
    f32 = np.float32
    bf16 = ml_dtypes.bfloat16
    x = np.asarray(inputs["x"]).astype(np.int64)
    emb = np.asarray(inputs["emb"], dtype=f32)
    pos = np.asarray(inputs["pos"], dtype=f32)

    positions = np.minimum(np.arange(T), L - 1)
    h0 = emb[x] + pos[positions][None, :, :]      # [B, T, E] fp32

    def to_dev_lhst(mat, kchunks, mcols, scale):
        m = _to_fp8(np.ascontiguousarray(mat) * scale)
        return m.reshape(kchunks, 128, mcols).transpose(1, 0, 2).reshape(
            128, kchunks * mcols)

    def to_dev_bf16(mat, kchunks, mcols):
        m = np.ascontiguousarray(mat).astype(bf16)
        return m.reshape(kchunks, 128, mcols).transpose(1, 0, 2).reshape(
            128, kchunks * mcols)

    wq = np.asarray(inputs["wq"], dtype=f32)
    wk = np.asarray(inputs["wk"], dtype=f32)
    wv = np.asarray(inputs["wv"], dtype=f32)
    pw = np.asarray(inputs["proj_w"], dtype=f32)
    f1 = np.asarray(inputs["ff_w1"], dtype=f32)
    f2 = np.asarray(inputs["ff_w2"], dtype=f32)

    wq_dev = np.stack([to_dev_lhst(wq[i].transpose(1, 0, 2).reshape(E, NH * HS),
                                   C, 512, WS_QK) for i in range(NB)])
    wk_dev = np.stack([to_dev_lhst(wk[i].transpose(1, 0, 2).reshape(E, NH * HS),
                                   C, 512, WS_QK) for i in range(NB)])
    wv_dev = np.stack([to_dev_bf16(wv[i].transpose(1, 0, 2).reshape(
        E, NH * HS), C, 512) for i in range(NB)])
    pw_dev = np.stack([to_dev_bf16(pw[i], C, 512) for i in range(NB)])
    f1_dev = np.stack([to_dev_bf16(f1[i], C, FF) for i in range(NB)])
    f2_dev = np.stack([to_dev_bf16(f2[i], CF, 512) for i in range(NB)])

    def vec_dev(v, chunks):
        return np.ascontiguousarray(v.astype(f32).reshape(chunks, 128).T)

    pb = np.asarray(inputs["proj_b"], dtype=f32)
    fb1 = np.asarray(inputs["ff_b1"], dtype=f32)
    fb2 = np.asarray(inputs["ff_b2"], dtype=f32)
    pb_zero = bool(np.all(pb == 0.0))
    fb2_zero = bool(np.all(fb2 == 0.0))
    pb_dev = np.concatenate([vec_dev(pb[i], C) for i in range(NB)], axis=1)
    fb1_dev = np.concatenate([vec_dev(fb1[i], CF)
                              for i in range(NB)], axis=1)
    fb2_dev = np.concatenate([vec_dev(fb2[i], C) for i in range(NB)], axis=1)
    ow_dev = np.ascontiguousarray(
        (np.asarray(inputs["out_w"], dtype=f32) / TEMP).astype(bf16)
    ).reshape(C, 128, V).transpose(1, 0, 2).reshape(128, C * V)
    ob_dev = (np.asarray(inputs["out_b"], dtype=f32) / TEMP).reshape(V, 1)
    tri_dev = np.triu(np.ones((128, 128), dtype=f32)).astype(bf16)

    gs, bs, ln_trivial = [], [], []
    for i in range(NB):
        for nm_g, nm_b in (("ln1_g", "ln1_b"), ("ln2_g", "ln2_b")):
            g = np.asarray(inputs[nm_g][i], dtype=f32)
            b = np.asarray(inputs[nm_b][i], dtype=f32)
            gs.append(vec_dev(g, C))
            bs.append(vec_dev(b, C))
            ln_trivial.append(bool(np.all(g == 1.0) and np.all(b == 0.0)))
    g = np.asarray(inputs["lnf_g"], dtype=f32)
    b = np.asarray(inputs["lnf_b"], dtype=f32)
    gs.append(vec_dev(g, C))
    bs.append(vec_dev(b, C))
    ln_trivial.append(bool(np.all(g == 1.0) and np.all(b == 0.0)))
    lng_dev = np.concatenate(gs, axis=1)
    lnb_dev = np.concatenate(bs, axis=1)

    shared = {
        "wq": wq_dev, "wk": wk_dev, "wv": wv_dev, "pw": pw_dev,
        "f1": f1_dev, "f2": f2_dev, "pb": pb_dev, "fb1": fb1_dev,
        "fb2": fb2_dev, "ow": ow_dev, "ob": ob_dev, "tri": tri_dev,
        "lng": lng_dev, "lnb": lnb_dev,
    }

    h0_cores = []
    for core in range(NCORES):
        hh = h0[SEQ * core:SEQ * core + SEQ]          # [SEQ, T, E]
        hT = hh.transpose(2, 0, 1).reshape(E, NTOK)   # [E, NTOK]
        h0_cores.append(np.ascontiguousarray(
            hT.reshape(C, 128, NTOK).transpose(1, 0, 2).reshape(
                128, C * NTOK).astype(bf16)))
    flags = (tuple(ln_trivial), pb_zero, fb2_zero)
    return shared, h0_cores, flags


def assemble_output(core_logits):
    """core_logits: list of [V, NTOK] fp32 -> [B, T, V]."""
    out = np.empty((B, T, V), np.float32)
    for core in range(NCORES):
        lg = core_logits[core].reshape(V, SEQ, T)
        out[SEQ * core:SEQ * core + SEQ] = lg.transpose(1, 2, 0)
    return out


def get_program(flags):
    if flags not in _PROGRAM_CACHE:
        ln_trivial, pb_zero, fb2_zero = flags
        _PROGRAM_CACHE[flags] = build_program(list(ln_trivial), pb_zero,
                                              fb2_zero)
    return _PROGRAM_CACHE[flags]


def reset_device():
    """Recover a wedged accelerator (axon session reset). Best-effort."""
    try:
        import ctypes
        import jax
        jax.devices()
        lib = ctypes.CDLL('/opt/axon/libaxon_pjrt.so')
        lib.axon_reset.restype = ctypes.c_int64
        lib.axon_reset()
    except Exception:
        pass


def kernel(**inputs):
    from concourse.bass_utils import run_bass_kernel_spmd
    shared, h0_cores, flags = prepare_inputs(inputs)
    nc = get_program(flags)
    in_maps = [dict(shared, h0=h0_cores[c]) for c in range(NCORES)]
    try:
        res = run_bass_kernel_spmd(nc, in_maps, core_ids=list(range(NCORES)))
    except Exception:
        # A previous (profiled) session can leave the device wedged; reset
        # the axon session and retry once.
        reset_device()
        res = run_bass_kernel_spmd(nc, in_maps, core_ids=list(range(NCORES)))
    return assemble_output([res.results[c]["logits"] for c in range(NCORES)])


# revision 27
# speedup vs baseline: 1.2929x; 1.2929x over previous
"""Trainium2 Bass kernel for nn_AutoregressiveArithmeticTransformer.

6-layer dense transformer: B=16, T=512, E=512, NH=8 heads x HS=64, FF=2048,
V=16, causal attention, pre-LN, learned abacus embedding, logits / 0.8.

Strategy: data-parallel over batch across 8 NeuronCores (2 sequences per
core, no collectives). Activations feature-major in SBUF; the residual
stream is bf16. All heavy matmuls (QKV/V/proj/FFN and the attn o/den
matmuls) run in fp8-e4m3 with the TensorE DoubleRow perf mode (2 k-tiles
of 128 per instruction, 2x bf16 throughput); weights are pre-scaled
host-side (x64 / x32) to sit in the fp8 normal range, and the inverse
scales are folded into downstream PSUM-eviction ops / the softmax exp
scale. Scores matmuls stay bf16 (K=64 cannot use DoubleRow). LayerNorm
statistics are ones-matmuls on the PE in bf16; softmax denominators are
ones-matmuls in fp8 sharing the causal block structure of the o matmul.
"""

import numpy as np
import ml_dtypes

import concourse.bacc as bacc
import concourse.tile as tile
from concourse import mybir

F32 = mybir.dt.float32
BF16 = mybir.dt.bfloat16
FP8 = mybir.dt.float8e4
AF = mybir.ActivationFunctionType
OP = mybir.AluOpType
DR = mybir.MatmulPerfMode.DoubleRow

E4 = ml_dtypes.float8_e4m3

# Model constants (hardcoded per contest contract)
V, E, NH, HS, FF, NB, L = 16, 512, 8, 64, 2048, 6, 512
B, T = 16, 512
TEMP = 1.0 * 0.8
EPS = 1e-5
SCALE = HS ** -0.5  # 0.125

NCORES = 8
SEQ = 2              # sequences per core
NTOK = SEQ * T       # 1024 tokens per core
C = E // 128         # 4 E-chunks
CF = FF // 128       # 16 FF-chunks
HP = NH // 2         # 4 head-pairs
NJ = T // 128        # 4 tk chunks per sequence

# fp8 weight pre-scales (keep weights in the e4m3 normal range)
WS_QK = 64.0         # wq, wk
WS_V = 64.0          # wv
WS_PW = 64.0         # proj_w
WS_F1 = 32.0         # ff_w1
WS_F2 = 64.0         # ff_w2
VT_SCALE = 8.0                 # vt8 = 8 * v_true
EXP_SCALE = SCALE / (WS_QK * WS_QK)
PROJ_RSCALE = 1.0 / (8.0 * WS_PW)      # o8 carries 8x
FFN_RSCALE = 1.0 / (WS_F1 * WS_F2)     # fa8 carries 32x

_PROGRAM_CACHE = {}


def _ceil2(x):
    return (x + 1) // 2 * 2


def build_program(ln_trivial, pb_zero, fb2_zero, nb_run=NB, dbg=False):
    """ln_trivial: list of 2*NB+1 bools; pb_zero/fb2_zero: biases all-zero."""
    nc = bacc.Bacc(None, target_bir_lowering=False)

    h0_d = nc.dram_tensor("h0", [128, C * NTOK], BF16, kind="ExternalInput")
    wq_d = nc.dram_tensor("wq", [NB, 128, C * 512], FP8, kind="ExternalInput")
    wk_d = nc.dram_tensor("wk", [NB, 128, C * 512], FP8, kind="ExternalInput")
    wv_d = nc.dram_tensor("wv", [NB, 128, C * 512], BF16, kind="ExternalInput")
    pw_d = nc.dram_tensor("pw", [NB, 128, C * 512], BF16, kind="ExternalInput")
    f1_d = nc.dram_tensor("f1", [NB, 128, C * FF], BF16, kind="ExternalInput")
    f2_d = nc.dram_tensor("f2", [NB, 128, CF * 512], BF16, kind="ExternalInput")
    pb_d = nc.dram_tensor("pb", [128, NB * C], F32, kind="ExternalInput")
    fb1_d = nc.dram_tensor("fb1", [128, NB * CF], F32, kind="ExternalInput")
    fb2_d = nc.dram_tensor("fb2", [128, NB * C], F32, kind="ExternalInput")
    ow_d = nc.dram_tensor("ow", [128, C * V], BF16, kind="ExternalInput")
    ob_d = nc.dram_tensor("ob", [V, 1], F32, kind="ExternalInput")
    tri_d = nc.dram_tensor("tri", [128, 128], BF16, kind="ExternalInput")
    lng_d = nc.dram_tensor("lng", [128, (2 * NB + 1) * C], F32,
                           kind="ExternalInput")
    lnb_d = nc.dram_tensor("lnb", [128, (2 * NB + 1) * C], F32,
                           kind="ExternalInput")
    out_d = nc.dram_tensor("logits", [V, NTOK], F32, kind="ExternalOutput")
    if dbg:
        xn_dbg_d = nc.dram_tensor("xn_dbg", [128, C * NTOK], F32,
                                  kind="ExternalOutput")
        o_dbg_d = nc.dram_tensor("o_dbg", [128, C * NTOK], F32,
                                 kind="ExternalOutput")
        h1_dbg_d = nc.dram_tensor("h1_dbg", [128, C * NTOK], F32,
                                  kind="ExternalOutput")
        vt_dbg_d = nc.dram_tensor("vt_dbg", [128, SEQ * NJ * NH * 128], FP8,
                                  kind="ExternalOutput")
        p_dbg_d = nc.dram_tensor("p_dbg", [128, 2 * NJ * 512], FP8,
                                 kind="ExternalOutput")
        ops_dbg_d = nc.dram_tensor("ops_dbg", [128, 2 * 512], F32,
                                   kind="ExternalOutput")

    from contextlib import ExitStack
    with ExitStack() as ctx:
        tc = ctx.enter_context(tile.TileContext(nc))
        consts = ctx.enter_context(tc.tile_pool(name="consts", bufs=1))
        hpool = ctx.enter_context(tc.tile_pool(name="hpool", bufs=1))
        wqkv = ctx.enter_context(tc.tile_pool(name="wqkv", bufs=2))
        wff1 = ctx.enter_context(tc.tile_pool(name="wff1", bufs=1 if dbg else 2))
        wff2 = ctx.enter_context(tc.tile_pool(name="wff2", bufs=1 if dbg else 2))
        xnp = ctx.enter_context(tc.tile_pool(name="xnp", bufs=2))
        sqp = ctx.enter_context(tc.tile_pool(name="sqp", bufs=1))
        qkp = ctx.enter_context(tc.tile_pool(name="qkp", bufs=2))
        pp = ctx.enter_context(tc.tile_pool(name="pp", bufs=3))
        osb = ctx.enter_context(tc.tile_pool(name="osb", bufs=1))
        ffa = ctx.enter_context(tc.tile_pool(name="ffa", bufs=1))
        stats = ctx.enter_context(tc.tile_pool(name="stats", bufs=2))
        lnt = ctx.enter_context(tc.tile_pool(name="lnt", bufs=2))
        dbgp = ctx.enter_context(tc.tile_pool(name="dbgp", bufs=1)) if dbg \
            else None
        rdp = ctx.enter_context(tc.tile_pool(name="rdp", bufs=2))
        # PSUM: ps1 = 1-bank [128,512] tiles; ps2 = 2-bank [128,2,512]
        ps1 = ctx.enter_context(tc.tile_pool(name="ps1", bufs=2, space="PSUM"))
        ps2 = ctx.enter_context(tc.tile_pool(name="ps2", bufs=3, space="PSUM"))

        ones_t = consts.tile([128, 128], BF16)
        nc.gpsimd.memset(ones_t[:], 1.0)
        ones8_t = consts.tile([128, 2, 128], FP8)
        nc.gpsimd.memset(ones8_t[:], 8.0)
        eps_t = consts.tile([128, 1], F32)
        nc.gpsimd.memset(eps_t[:], float(EPS))
        tri_t = consts.tile([128, 128], BF16)
        nc.sync.dma_start(tri_t[:], tri_d[:])
        pb_t = consts.tile([128, NB * C], F32)
        nc.sync.dma_start(pb_t[:], pb_d[:])
        fb1_t = consts.tile([128, NB * CF], F32)
        nc.sync.dma_start(fb1_t[:], fb1_d[:])
        fb2_t = consts.tile([128, NB * C], F32)
        nc.sync.dma_start(fb2_t[:], fb2_d[:])
        ow_t = consts.tile([128, C, V], BF16)
        nc.sync.dma_start(ow_t[:], ow_d[:].rearrange("p (c v) -> p c v", v=V))
        ob_t = consts.tile([V, 1], F32)
        nc.sync.dma_start(ob_t[:], ob_d[:])
        lng_t = consts.tile([128, 2 * NB + 1, C], F32)
        nc.sync.dma_start(lng_t[:], lng_d[:].rearrange("p (l c) -> p l c", c=C))
        lnb_t = consts.tile([128, 2 * NB + 1, C], F32)
        nc.sync.dma_start(lnb_t[:], lnb_d[:].rearrange("p (l c) -> p l c", c=C))

        h_t = hpool.tile([128, C, NTOK], BF16)
        # V values augmented with a ones column-block per head: the o-matmul
        # lhsT [v64 | ones64] produces o on partitions 0:64 and the softmax
        # denominator on 64:128 of the same PSUM tile (DoubleRow needs
        # partition-base-0 outputs, so den cannot be tile-positioned).
        vt8aug = hpool.tile([128, SEQ * NJ, NH, 128], FP8)
        nc.gpsimd.memset(vt8aug[:, :, 0:NH:2, 64:128], 1.0)
        nc.gpsimd.memset(vt8aug[:, :, 1:NH:2, 0:64], 1.0)
        nc.sync.dma_start(h_t[:], h0_d[:].rearrange("p (c t) -> p c t",
                                                    t=NTOK))

        def ln_params(idx):
            if not ln_trivial[idx]:
                return lng_t[:, idx, :], lnb_t[:, idx, :], False
            return None, None, True

        # ---- LayerNorm: one token-tile (512 cols) ----
        def emit_ln_tt(tt, g_ap, b_ap, triv, xn, out_fp8):
            sl = slice(tt * 512, tt * 512 + 512)
            sq = sqp.tile([128, C, 512], BF16, tag="sq", name="sq")
            nc.vector.tensor_tensor(sq[:], h_t[:, :, sl], h_t[:, :, sl],
                                    OP.mult)
            s1 = ps1.tile([128, 512], F32, tag="ps1", name="s1")
            s2 = ps1.tile([128, 512], F32, tag="ps1", name="s2")
            for c in range(C):
                nc.tensor.matmul(s1[:], ones_t[:], h_t[:, c, sl],
                                 start=(c == 0), stop=(c == C - 1))
                nc.tensor.matmul(s2[:], ones_t[:], sq[:, c, :],
                                 start=(c == 0), stop=(c == C - 1))
            m_bf = stats.tile([128, 512], BF16, tag="m", name="m_bf")
            nc.scalar.mul(m_bf[:], s1[:], 1.0 / E)
            msq = stats.tile([128, 512], BF16, tag="msq", name="msq")
            nc.scalar.square(msq[:], m_bf[:])
            var = stats.tile([128, 512], F32, tag="var", name="var")
            nc.vector.scalar_tensor_tensor(out=var[:], in0=s2[:],
                                           scalar=1.0 / E, in1=msq[:],
                                           op0=OP.mult, op1=OP.subtract)
            std = stats.tile([128, 512], F32, tag="std", name="std")
            nc.scalar.activation(std[:], var[:], AF.Sqrt, bias=eps_t[:])
            rc = stats.tile([128, 512], F32, tag="rc", name="rc")
            nc.vector.reciprocal_approx_fast(out=rc[:], in_=std[:])
            t1 = lnt.tile([128, C, 512], BF16, tag="t1", name="t1")
            nc.vector.tensor_tensor(
                t1[:], h_t[:, :, sl],
                m_bf[:, None, :].to_broadcast((128, C, 512)), OP.subtract)
            if triv:
                nc.vector.tensor_tensor(
                    xn[:, :, sl], t1[:],
                    rc[:, None, :].to_broadcast((128, C, 512)), OP.mult)
            else:
                xb = lnt.tile([128, C, 512], BF16, tag="xb", name="xb")
                nc.vector.tensor_tensor(
                    xb[:], t1[:],
                    rc[:, None, :].to_broadcast((128, C, 512)), OP.mult)
                for c in range(C):
                    nc.vector.tensor_scalar(
                        out=xn[:, c, sl], in0=xb[:, c, :],
                        scalar1=g_ap[:, c:c + 1], scalar2=b_ap[:, c:c + 1],
                        op0=OP.mult, op1=OP.add)

        def emit_ln(idx, out_fp8=True):
            g_ap, b_ap, triv = ln_params(idx)
            xn = xnp.tile([128, C, NTOK], BF16, tag="xn", name="xn")
            for tt in range(2):
                emit_ln_tt(tt, g_ap, b_ap, triv, xn, out_fp8)
            return xn

        for i in range(nb_run):
            # ---- load this layer's weights (fp8) ----
            wq_t = wqkv.tile([128, C, 512], FP8, tag="wq", name="wq_t")
            nc.sync.dma_start(wq_t[:], wq_d[i].rearrange(
                "p (c m) -> p c m", m=512))
            wk_t = wqkv.tile([128, C, 512], FP8, tag="wk", name="wk_t")
            nc.sync.dma_start(wk_t[:], wk_d[i].rearrange(
                "p (c m) -> p c m", m=512))
            wv_t = wqkv.tile([128, C, 512], BF16, tag="wv", name="wv_t")
            nc.sync.dma_start(wv_t[:], wv_d[i].rearrange(
                "p (c m) -> p c m", m=512))
            pw_t = wqkv.tile([128, C, 512], BF16, tag="pw", name="pw_t")
            nc.sync.dma_start(pw_t[:], pw_d[i].rearrange(
                "p (c m) -> p c m", m=512))
            f1_t = wff1.tile([128, C, FF], BF16, tag="f1", name="f1_t")
            nc.sync.dma_start(f1_t[:], f1_d[i].rearrange(
                "p (c m) -> p c m", m=FF))
            f2_t = wff2.tile([128, CF, 512], BF16, tag="f2", name="f2_t")
            nc.sync.dma_start(f2_t[:], f2_d[i].rearrange(
                "p (c m) -> p c m", m=512))

            if i == 0:
                xn = emit_ln(0, out_fp8=False)
            else:
                xn = xn_next

            # ---- V projection, token-major: vt8aug[tok, head, 0:64] = 8*v
            for jg in range(SEQ * NJ):
                vp = ps1.tile([128, 512], F32, tag="ps1", name="vp")
                for c in range(C):
                    nc.tensor.matmul(vp[:],
                                     xn[:, c, jg * 128:(jg + 1) * 128],
                                     wv_t[:, c, :],
                                     start=(c == 0), stop=(c == C - 1))
                vpr = vp[:].rearrange("p (hp h2 d) -> p hp h2 d",
                                      h2=2, d=64)
                nc.scalar.mul(vt8aug[:, jg, 0:NH:2, 0:64], vpr[:, :, 0, :],
                              VT_SCALE)
                nc.scalar.mul(vt8aug[:, jg, 1:NH:2, 64:128], vpr[:, :, 1, :],
                              VT_SCALE)

            o_t = osb.tile([128, C, NTOK], BF16, tag="o", name="o_t")

            def emit_den_o(s, hp, p_t):
                base = s * T
                # o matmul: lhsT [v|ones] (even head) / [ones|v] (odd) so
                # each head's o lands on its destination partition half
                # (DVE reads are partition-aligned with the out base, and
                # DoubleRow outputs must start at partition 0).
                ops = ps2.tile([128, 2, 512], F32, tag="ps2", name="ops")
                den = ps2.tile([128, 2, 512], F32, tag="ps2", name="den")
                for h2 in range(2):
                    head = hp * 2 + h2
                    for dst, lh in ((ops, lambda j: vt8aug[:, s * NJ + j:
                                                          s * NJ + j + 2,
                                                          head, :]),
                                    (den, lambda j: ones8_t[:])):
                        nc.tensor.matmul(dst[:, h2, :], lh(0),
                                         p_t[:, h2, 0:2, :],
                                         start=True, stop=False,
                                         perf_mode=DR)
                        nc.tensor.matmul(dst[:, h2, 256:512], lh(2),
                                         p_t[:, h2, 2:4, 256:512],
                                         start=False, stop=True,
                                         perf_mode=DR)
                if dbg and s == 0 and hp == 0:
                    opsc = dbgp.tile([128, C, NTOK], F32, tag="dbg", name="opsc")[:, 0:2, 0:512]
                    nc.scalar.copy(opsc, ops[:])
                    nc.sync.dma_start(ops_dbg_d[:].rearrange(
                        "p (a t) -> p a t", a=2), opsc)
                    nc.sync.dma_start(p_dbg_d[:], p_t[:].rearrange(
                        "p a j t -> p (a j t)"))
                    nc.sync.dma_start(vt_dbg_d[:], vt8aug[:].rearrange(
                        "p j h d -> p (j h d)"))
                rd = rdp.tile([128, 2, 512], F32, tag="rd", name="rd")
                nc.vector.reciprocal_approx_fast(out=rd[:], in_=den[:])
                for h2 in range(2):
                    dsl = slice(h2 * 64, h2 * 64 + 64)
                    nc.vector.tensor_tensor(
                        o_t[dsl, hp, base:base + T],
                        ops[dsl, h2, :], rd[dsl, h2, :], OP.mult)

            pending = None
            for hp in range(HP):
                msl = slice(hp * 128, (hp + 1) * 128)
                qk_t = qkp.tile([128, 2, NTOK], BF16, tag="qk", name="qk_t")
                for tt in range(2):
                    sl = slice(tt * 512, tt * 512 + 512)
                    qkps = ps2.tile([128, 2, 512], F32, tag="ps2", name="qkps")
                    for c in range(C):
                        nc.tensor.matmul(qkps[:, 0, :], wq_t[:, c, msl],
                                         xn[:, c, sl],
                                         start=(c == 0), stop=(c == C - 1))
                        nc.tensor.matmul(qkps[:, 1, :], wk_t[:, c, msl],
                                         xn[:, c, sl],
                                         start=(c == 0), stop=(c == C - 1))
                    nc.vector.tensor_copy(qk_t[:, :, sl], qkps[:])

                for s in range(SEQ):
                    base = s * T
                    p_t = pp.tile([128, 2, NJ, 512], FP8, tag="p", name="p_t")
                    nc.vector.memset(p_t[:, :, 1, 0:128], 0.0)
                    nc.vector.memset(p_t[:, :, 3, 256:384], 0.0)
                    for j in range(NJ):
                        off = j * 128
                        njw = T - off
                        sT = ps2.tile([128, 2, 512], F32, tag="ps2",
                                      name="sT")
                        for h2 in range(2):
                            dsl = slice(h2 * 64, h2 * 64 + 64)
                            nc.tensor.matmul(
                                sT[:, h2, off:512],
                                qk_t[dsl, 1, base + off:base + off + 128],
                                qk_t[dsl, 0, base + off:base + T],
                                start=True, stop=True)
                        nc.scalar.activation(
                            p_t[:, :, j, off:512], sT[:, :, off:512],
                            AF.Exp, scale=EXP_SCALE)
                        nc.vector.tensor_tensor(
                            p_t[:, :, j, off:off + 128],
                            p_t[:, :, j, off:off + 128],
                            tri_t[:, None, :].to_broadcast((128, 2, 128)),
                            OP.mult)
                    if pending is not None:
                        emit_den_o(*pending)
                    pending = (s, hp, p_t)
            emit_den_o(*pending)

            if dbg and i == 0:
                dcp = dbgp.tile([128, C, NTOK], F32, tag="dbg", name="dcp")
                nc.scalar.copy(dcp[:], o_t[:])
                nc.sync.dma_start(o_dbg_d[:], dcp[:].rearrange(
                    "p c t -> p (c t)"))

            # ---- attention out projection + residual ----
            for tt in range(2):
                sl = slice(tt * 512, tt * 512 + 512)
                for mcp in range(2):
                    pj = ps2.tile([128, 2, 512], F32, tag="ps2", name="pj")
                    for mc2 in range(2):
                        mc = 2 * mcp + mc2
                        for c in range(C):
                            nc.tensor.matmul(
                                pj[:, mc2, :],
                                pw_t[:, c, mc * 128:(mc + 1) * 128],
                                o_t[:, c, sl],
                                start=(c == 0), stop=(c == C - 1))
                    if pb_zero:
                        nc.vector.scalar_tensor_tensor(
                            out=h_t[:, 2 * mcp:2 * mcp + 2, sl],
                            in0=pj[:], scalar=1.0,
                            in1=h_t[:, 2 * mcp:2 * mcp + 2, sl],
                            op0=OP.mult, op1=OP.add)
                    else:
                        tmp = stats.tile([128, 2, 512], BF16, tag="tmp",
                                         name="tmp")
                        for mc2 in range(2):
                            mc = 2 * mcp + mc2
                            nc.vector.tensor_scalar(
                                out=tmp[:, mc2, :], in0=pj[:, mc2, :],
                                scalar1=1.0,
                                scalar2=pb_t[:, i * C + mc:i * C + mc + 1],
                                op0=OP.mult, op1=OP.add)
                        nc.vector.tensor_tensor(
                            h_t[:, 2 * mcp:2 * mcp + 2, sl], tmp[:],
                            h_t[:, 2 * mcp:2 * mcp + 2, sl], OP.add)

            if dbg and i == 0:
                dcp3 = dbgp.tile([128, C, NTOK], F32, tag="dbg", name="dcp3")
                nc.scalar.copy(dcp3[:], h_t[:])
                nc.sync.dma_start(h1_dbg_d[:], dcp3[:].rearrange(
                    "p c t -> p (c t)"))

            # ---- LN2 + FFN (token-tile split) ----
            xn2 = emit_ln(2 * i + 1, out_fp8=False)

            if i + 1 < nb_run:
                g_next, b_next, triv_next = ln_params(2 * (i + 1))
                xn_next = xnp.tile([128, C, NTOK], BF16, tag="xn",
                                   name="xn_next")

            for tt in range(2):
                sl = slice(tt * 512, tt * 512 + 512)
                fa = ffa.tile([128, CF, 512], BF16, tag="fa", name="fa")
                for mfp in range(CF // 2):
                    fp = ps2.tile([128, 2, 512], F32, tag="ps2", name="fp")
                    for mf2 in range(2):
                        mf = 2 * mfp + mf2
                        for c in range(C):
                            nc.tensor.matmul(
                                fp[:, mf2, :],
                                f1_t[:, c, mf * 128:(mf + 1) * 128],
                                xn2[:, c, sl],
                                start=(c == 0), stop=(c == C - 1))
                    for mf2 in range(2):
                        mf = 2 * mfp + mf2
                        nc.scalar.activation(
                            fa[:, mf, :], fp[:, mf2, :], AF.Relu,
                            bias=fb1_t[:, i * CF + mf:i * CF + mf + 1])
                for mcp in range(2):
                    f2p = ps2.tile([128, 2, 512], F32, tag="ps2", name="f2p")
                    for mc2 in range(2):
                        mc = 2 * mcp + mc2
                        for c16 in range(CF):
                            nc.tensor.matmul(
                                f2p[:, mc2, :],
                                f2_t[:, c16, mc * 128:(mc + 1) * 128],
                                fa[:, c16, :],
                                start=(c16 == 0), stop=(c16 == CF - 1))
                    if fb2_zero:
                        nc.vector.scalar_tensor_tensor(
                            out=h_t[:, 2 * mcp:2 * mcp + 2, sl],
                            in0=f2p[:], scalar=1.0,
                            in1=h_t[:, 2 * mcp:2 * mcp + 2, sl],
                            op0=OP.mult, op1=OP.add)
                    else:
                        tmp = stats.tile([128, 2, 512], BF16, tag="tmp",
                                         name="tmp2")
                        for mc2 in range(2):
                            mc = 2 * mcp + mc2
                            nc.vector.tensor_scalar(
                                out=tmp[:, mc2, :], in0=f2p[:, mc2, :],
                                scalar1=1.0,
                                scalar2=fb2_t[:, i * C + mc:i * C + mc + 1],
                                op0=OP.mult, op1=OP.add)
                        nc.vector.tensor_tensor(
                            h_t[:, 2 * mcp:2 * mcp + 2, sl], tmp[:],
                            h_t[:, 2 * mcp:2 * mcp + 2, sl], OP.add)
                # peel next layer's LN1 token-tile
                if i + 1 < nb_run:
                    emit_ln_tt(tt, g_next, b_next, triv_next, xn_next, True)

        # ---- final LN + logits ----
        xnf = emit_ln(2 * NB if nb_run == NB else 0, out_fp8=False)
        lg_sb = consts.tile([V, NTOK], F32)
        for tt in range(2):
            sl = slice(tt * 512, tt * 512 + 512)
            lg = ps1.tile([V, 512], F32, tag="ps1", name="lg")
            for c in range(C):
                nc.tensor.matmul(lg[:], ow_t[:, c, :], xnf[:, c, sl],
                                 start=(c == 0), stop=(c == C - 1))
            nc.vector.tensor_scalar_add(lg_sb[:, sl], lg[:], ob_t[:])
        nc.sync.dma_start(out_d[:], lg_sb[:])

    nc.finalize()
    return nc


def _to_fp8(x):
    return np.clip(x, -240.0, 240.0).astype(E4)


def prepare_inputs(inputs):
    """Host-side preprocessing: embedding gather, weight layout + fp8 cast.
    Returns (shared_map, per_core_h0_list, flags)."""
    f32 = np.float32
    bf16 = ml_dtypes.bfloat16
    x = np.asarray(inputs["x"]).astype(np.int64)
    emb = np.asarray(inputs["emb"], dtype=f32)
    pos = np.asarray(inputs["pos"], dtype=f32)

    positions = np.minimum(np.arange(T), L - 1)
    h0 = emb[x] + pos[positions][None, :, :]      # [B, T, E] fp32

    def to_dev_lhst(mat, kchunks, mcols, scale):
        m = _to_fp8(np.ascontiguousarray(mat) * scale)
        return m.reshape(kchunks, 128, mcols).transpose(1, 0, 2).reshape(
            128, kchunks * mcols)

    def to_dev_bf16(mat, kchunks, mcols):
        m = np.ascontiguousarray(mat).astype(bf16)
        return m.reshape(kchunks, 128, mcols).transpose(1, 0, 2).reshape(
            128, kchunks * mcols)

    wq = np.asarray(inputs["wq"], dtype=f32)
    wk = np.asarray(inputs["wk"], dtype=f32)
    wv = np.asarray(inputs["wv"], dtype=f32)
    pw = np.asarray(inputs["proj_w"], dtype=f32)
    f1 = np.asarray(inputs["ff_w1"], dtype=f32)
    f2 = np.asarray(inputs["ff_w2"], dtype=f32)

    wq_dev = np.stack([to_dev_lhst(wq[i].transpose(1, 0, 2).reshape(E, NH * HS),
                                   C, 512, WS_QK) for i in range(NB)])
    wk_dev = np.stack([to_dev_lhst(wk[i].transpose(1, 0, 2).reshape(E, NH * HS),
                                   C, 512, WS_QK) for i in range(NB)])
    wv_dev = np.stack([to_dev_bf16(wv[i].transpose(1, 0, 2).reshape(
        E, NH * HS), C, 512) for i in range(NB)])
    pw_dev = np.stack([to_dev_bf16(pw[i], C, 512) for i in range(NB)])
    f1_dev = np.stack([to_dev_bf16(f1[i], C, FF) for i in range(NB)])
    f2_dev = np.stack([to_dev_bf16(f2[i], CF, 512) for i in range(NB)])

    def vec_dev(v, chunks):
        return np.ascontiguousarray(v.astype(f32).reshape(chunks, 128).T)

    pb = np.asarray(inputs["proj_b"], dtype=f32)
    fb1 = np.asarray(inputs["ff_b1"], dtype=f32)
    fb2 = np.asarray(inputs["ff_b2"], dtype=f32)
    pb_zero = bool(np.all(pb == 0.0))
    fb2_zero = bool(np.all(fb2 == 0.0))
    pb_dev = np.concatenate([vec_dev(pb[i], C) for i in range(NB)], axis=1)
    fb1_dev = np.concatenate([vec_dev(fb1[i], CF)
                              for i in range(NB)], axis=1)
    fb2_dev = np.concatenate([vec_dev(fb2[i], C) for i in range(NB)], axis=1)
    ow_dev = np.ascontiguousarray(
        (np.asarray(inputs["out_w"], dtype=f32) / TEMP).astype(bf16)
    ).reshape(C, 128, V).transpose(1, 0, 2).reshape(128, C * V)
    ob_dev = (np.asarray(inputs["out_b"], dtype=f32) / TEMP).reshape(V, 1)
    tri_dev = np.triu(np.ones((128, 128), dtype=f32)).astype(bf16)

    gs, bs, ln_trivial = [], [], []
    for i in range(NB):
        for nm_g, nm_b in (("ln1_g", "ln1_b"), ("ln2_g", "ln2_b")):
            g = np.asarray(inputs[nm_g][i], dtype=f32)
            b = np.asarray(inputs[nm_b][i], dtype=f32)
            gs.append(vec_dev(g, C))
            bs.append(vec_dev(b, C))
            ln_trivial.append(bool(np.all(g == 1.0) and np.all(b == 0.0)))
    g = np.asarray(inputs["lnf_g"], dtype=f32)
    b = np.asarray(inputs["lnf_b"], dtype=f32)
    gs.append(vec_dev(g, C))
    bs.append(vec_dev(b, C))
    ln_trivial.append(bool(np.all(g == 1.0) and np.all(b == 0.0)))
    lng_dev = np.concatenate(gs, axis=1)
    lnb_dev = np.concatenate(bs, axis=1)

    shared = {
        "wq": wq_dev, "wk": wk_dev, "wv": wv_dev, "pw": pw_dev,
        "f1": f1_dev, "f2": f2_dev, "pb": pb_dev, "fb1": fb1_dev,
        "fb2": fb2_dev, "ow": ow_dev, "ob": ob_dev, "tri": tri_dev,
        "lng": lng_dev, "lnb": lnb_dev,
    }

    h0_cores = []
    for core in range(NCORES):
        hh = h0[SEQ * core:SEQ * core + SEQ]          # [SEQ, T, E]
        hT = hh.transpose(2, 0, 1).reshape(E, NTOK)   # [E, NTOK]
        h0_cores.append(np.ascontiguousarray(
            hT.reshape(C, 128, NTOK).transpose(1, 0, 2).reshape(
                128, C * NTOK).astype(bf16)))
    flags = (tuple(ln_trivial), pb_zero, fb2_zero)
    return shared, h0_cores, flags


def assemble_output(core_logits):
    """core_logits: list of [V, NTOK] fp32 -> [B, T, V]."""
    out = np.empty((B, T, V), np.float32)
    for core in range(NCORES):
        lg = core_logits[core].reshape(V, SEQ, T)
        out[SEQ * core:SEQ * core + SEQ] = lg.transpose(1, 2, 0)
    return out


def get_program(flags):
    if flags not in _PROGRAM_CACHE:
        ln_trivial, pb_zero, fb2_zero = flags
        _PROGRAM_CACHE[flags] = build_program(list(ln_trivial), pb_zero,
                                              fb2_zero)
    return _PROGRAM_CACHE[flags]


def reset_device():
    """Recover a wedged accelerator (axon session reset). Best-effort."""
    try:
        import ctypes
        import jax
        jax.devices()
        lib = ctypes.CDLL('/opt/axon/libaxon_pjrt.so')
        lib.axon_reset.restype = ctypes.c_int64
        lib.axon_reset()
    except Exception:
        pass


def kernel(**inputs):
    from concourse.bass_utils import run_bass_kernel_spmd
    shared, h0_cores, flags = prepare_inputs(inputs)
    nc = get_program(flags)
    in_maps = [dict(shared, h0=h0_cores[c]) for c in range(NCORES)]
    try:
        res = run_bass_kernel_spmd(nc, in_maps, core_ids=list(range(NCORES)))
    except Exception:
        # A previous (profiled) session can leave the device wedged; reset
        # the axon session and retry once.
        reset_device()
        res = run_bass_kernel_spmd(nc, in_maps, core_ids=list(range(NCORES)))
    return assemble_output([res.results[c]["logits"] for c in range(NCORES)])
